# revision 1
# baseline (speedup 1.0000x reference)
"""Trainium2 Bass kernel for nn_Attention_b (tanh-attention with masked_scatter).

Data-parallel over batch: each of 8 NeuronCores owns 4 batches. Per core:
  phase 1  z = W1 @ h_i + (W2 @ h_t + b)   (fp32r GEMM, [A, rows])
           m = tanh(z); y = u . m          (raw scores, [rows])
  comm     AllGather of the per-chunk score slice across the 8 cores
  phase 2  masked_scatter selection (0/1 matrix against gathered scores)
           + online softmax over the sequence (flash-style, chunked)
  phase 3  s_acc += e * h_i  (fused DVE multiply-reduce on the resident
           h_i chunk -- h_i is read from HBM exactly once)
"""
import sys

for _p in ("/opt/trn_rl_repo",):
    if _p not in sys.path:
        sys.path.insert(0, _p)

import numpy as np

import concourse.bacc as bacc
import concourse.tile as tile
from concourse import mybir
from concourse.bass_utils import run_bass_kernel_spmd
from concourse.dve_ops import TENSOR_TENSOR_REDUCE
from concourse.masks import make_identity

NCORES = 8
B, S, H, A = 32, 2048, 1024, 256
BL = B // NCORES          # local batches per core
NEG = np.float32(-1e20)

f32 = mybir.dt.float32
f32r = mybir.dt.float32r


def build_kernel(S=S, H=H, A=A, C=256, hi_bufs=4, clist=None):
    KT = H // 128             # contraction tiles
    AT = A // 128             # score tiles
    if clist is None:
        clist = [C] * (S // C)
    offs = np.concatenate([[0], np.cumsum(clist)]).tolist()
    NCH = len(clist)
    assert offs[-1] == S and H % 128 == 0 and A % 128 == 0

    nc = bacc.Bacc("TRN2", target_bir_lowering=False, debug=False,
                   num_devices=NCORES)

    # big operands are declared float32r (same bits as f32) so the plain
    # HWDGE DMA path can be used -- no SWDGE cast, no Q7 descriptor work
    hi5 = nc.declare_dram_parameter("hi5", [128, KT * BL * S], f32r,
                                    isOutput=False)
    w1t = nc.declare_dram_parameter("w1t", [H, A], f32r, isOutput=False)
    cb2 = nc.declare_dram_parameter("cb2", [128, AT, BL], f32, isOutput=False)
    u2 = nc.declare_dram_parameter("u2", [128, AT], f32r, isOutput=False)
    sel = nc.declare_dram_parameter("sel", [B + 1, BL, S], f32,
                                    isOutput=False)
    out = nc.declare_dram_parameter("out", [BL, H], f32, isOutput=True)

    with tile.TileContext(nc) as tc:
        with (
            tc.tile_pool(name="consts", bufs=1) as cp,
            tc.tile_pool(name="hi", bufs=hi_bufs) as hip,
            tc.tile_pool(name="m", bufs=2) as mp,
            tc.tile_pool(name="small", bufs=3) as sp,
            tc.tile_pool(name="ebc", bufs=2) as ebp,
            tc.tile_pool(name="sacc", bufs=2) as sap,
            tc.tile_pool(name="pz", bufs=2, space="PSUM") as pz,
            tc.tile_pool(name="py", bufs=2, space="PSUM") as py,
            tc.tile_pool(name="dram", bufs=NCH, space="DRAM") as dp,
        ):
            # ---- preload replicated constants
            w1_sb = cp.tile([128, KT, A], f32r)
            nc.sync.dma_start(
                out=w1_sb, in_=w1t.rearrange("(t p) a -> p t a", p=128))
            u_sb = cp.tile([128, AT], f32r)
            nc.sync.dma_start(out=u_sb, in_=u2[:, :])
            cb_sb = cp.tile([128, AT, BL], f32)
            nc.sync.dma_start(out=cb_sb, in_=cb2[:, :, :])
            ident = cp.tile([128, 128], f32)
            make_identity(nc, ident)
            ones_sb = cp.tile([B + 1, 1], f32r)
            nc.vector.memset(ones_sb.bitcast(f32), 1.0)

            # ---- per-chunk softmax stats (combined once at the end)
            mall = cp.tile([1, BL, NCH], f32)
            lall = cp.tile([1, BL, NCH], f32)
            saccs = [cp.tile([128, KT, BL], f32, name=f"sacc{i}")
                     for i in range(NCH)]


            carries = []

            def phase1(i):
                Ci, off = clist[i], offs[i]
                hi_sb = hip.tile([128, KT, BL, Ci], f32r, tag="hi")
                nc.sync.dma_start(
                    out=hi_sb.rearrange("p t b s -> p (t b s)"),
                    in_=hi5[:, KT * BL * off : KT * BL * (off + Ci)])
                sel_c = sp.tile([B + 1, BL, Ci], f32, tag="selc")
                nc.scalar.dma_start(out=sel_c, in_=sel[:, :, off : off + Ci])
                m_r = mp.tile([128, AT, BL, Ci], f32r, tag="m")
                for at in range(AT):
                    z_ps = pz.tile([128, BL, Ci], f32, tag="z")
                    for r in range(BL // 2):
                        for kt in range(KT):
                            nc.tensor.matmul(
                                z_ps[:, 2 * r : 2 * r + 2, :],
                                w1_sb[:, kt, at * 128 : (at + 1) * 128],
                                hi_sb[:, kt, 2 * r : 2 * r + 2, :],
                                start=(kt == 0), stop=(kt == KT - 1),
                            )
                    for b in range(BL):
                        nc.scalar.activation(
                            out=m_r[:, at, b, :], in_=z_ps[:, b, :],
                            func=mybir.ActivationFunctionType.Tanh,
                            bias=cb_sb[:, at, b : b + 1], scale=1.0,
                        )
                y_ps = py.tile([1, BL, Ci], f32, tag="y")
                for r in range(BL // 2):
                    for at in range(AT):
                        nc.tensor.matmul(
                            y_ps[:, 2 * r : 2 * r + 2, :],
                            u_sb[:, at : at + 1],
                            m_r[:, at, 2 * r : 2 * r + 2, :],
                            start=(at == 0), stop=(at == AT - 1),
                        )
                y_sb = sp.tile([1, BL, Ci], f32, tag="ysb", bufs=2)
                nc.scalar.activation(out=y_sb, in_=y_ps,
                                     func=mybir.ActivationFunctionType.Copy)

                ag_in = dp.tile([BL * Ci], f32, tag="agin")
                nc.scalar.dma_start(
                    out=ag_in.rearrange("(o n) -> o n", o=1),
                    in_=y_sb.rearrange("p b s -> p (b s)"))
                ag_out = dp.tile([B * Ci], f32, tag="agout",
                                 addr_space="Shared")
                nc.gpsimd.collective_compute(
                    "AllGather", mybir.AluOpType.bypass,
                    ins=[ag_in[:]], outs=[ag_out[:]],
                    replica_groups=[list(range(NCORES))],
                )
                y32 = sp.tile([B + 1, Ci], f32, tag="y32")
                nc.gpsimd.memset(y32[B : B + 1, :], 1.0)
                nc.scalar.dma_start(
                    out=y32[:B, :], in_=ag_out.rearrange("(j s) -> j s", s=Ci))
                return dict(hi_sb=hi_sb, sel_c=sel_c, y32=y32, i=i, Ci=Ci)

            def phase2(c):
                i, Ci = c["i"], c["Ci"]
                sel_c, y32 = c["sel_c"], c["y32"]
                # masked_scatter selection: one-hot rows (plus a -1e20 mask
                # row) dotted with [y; 1]
                bt_ps = py.tile([1, BL, Ci], f32, tag="y")
                selY = sp.tile([B + 1, BL, Ci], f32r, tag="selY", bufs=2)
                nc.vector.tensor_mul(
                    selY, sel_c,
                    y32.rearrange("j (o s) -> j o s", o=1)
                       .broadcast_to([B + 1, BL, Ci]))
                for hf in range(2):
                    nc.tensor.matmul(
                        bt_ps[:, 2 * hf : 2 * hf + 2, :], ones_sb,
                        selY[:, 2 * hf : 2 * hf + 2, :],
                        start=True, stop=True)

                # chunk-local max -> no cross-chunk recurrence
                cmax = sp.tile([1, BL], f32, tag="cmax")
                nc.vector.tensor_reduce(
                    out=cmax.rearrange("p (b o) -> p b o", o=1), in_=bt_ps,
                    axis=mybir.AxisListType.X, op=mybir.AluOpType.max)
                nc.vector.tensor_copy(mall[:, :, i], cmax)
                nmnew = sp.tile([1, BL], f32, tag="nmnew")
                nc.vector.tensor_scalar_mul(nmnew, cmax, -1.0)
                e4 = sp.tile([1, BL, Ci], f32, tag="e4", bufs=2)
                for b in range(BL):
                    nc.scalar.activation(
                        out=e4[:, b, :], in_=bt_ps[:, b, :],
                        func=mybir.ActivationFunctionType.Exp,
                        bias=nmnew[:, b : b + 1], scale=1.0,
                        accum_out=lall[:, b, i : i + 1])
                e_bc = ebp.tile([128, BL, Ci], f32, tag="ebc")
                nc.gpsimd.partition_broadcast(
                    e_bc.rearrange("p b s -> p (b s)"),
                    e4.rearrange("p b s -> p (b s)"))
                c["ebc"] = e_bc

            def phase3(c):
                i, Ci = c["i"], c["Ci"]
                sacc_i = saccs[i]
                ttr_scr = sp.tile([128, 1], f32, tag="ttrscr")
                hi_sb = c["hi_sb"]
                e_bc_all = c["ebc"]
                for b in range(BL):
                    e_bc = e_bc_all[:, b, :]
                    for kt in range(KT):
                        nc.vector._custom_dve(
                            TENSOR_TENSOR_REDUCE,
                            out=ttr_scr.broadcast_to([128, Ci]),
                            in0=hi_sb[:, kt, b, :].bitcast(f32),
                            in1=e_bc,
                            s0=0.0, s1=1.0,
                            accum_out=sacc_i[:, kt, b : b + 1],
                        )

            for i in range(NCH):
                carries.append(phase1(i))
                if len(carries) >= 2:
                    phase2(carries[-2])
                if len(carries) >= 3:
                    phase3(carries.pop(0))
            phase2(carries[-1])
            while carries:
                phase3(carries.pop(0))

            # ---- finalize: combine chunk partials, divide, transpose, store
            M = sp.tile([1, BL], f32, tag="cmax")
            nc.vector.tensor_reduce(
                out=M.rearrange("p (b o) -> p b o", o=1), in_=mall,
                axis=mybir.AxisListType.X, op=mybir.AluOpType.max)
            nM = sp.tile([1, BL], f32, tag="nmnew")
            nc.vector.tensor_scalar_mul(nM, M, -1.0)
            w = sp.tile([1, BL, NCH], f32, tag="w")
            for b in range(BL):
                nc.scalar.activation(
                    out=w[:, b, :], in_=mall[:, b, :],
                    func=mybir.ActivationFunctionType.Exp,
                    bias=nM[:, b : b + 1], scale=1.0)
            wl = sp.tile([1, BL, NCH], f32, tag="wl")
            nc.vector.tensor_mul(wl, w, lall)
            lsum = sp.tile([1, BL], f32, tag="lsum")
            nc.vector.tensor_reduce(
                out=lsum.rearrange("p (b o) -> p b o", o=1), in_=wl,
                axis=mybir.AxisListType.X, op=mybir.AluOpType.add)
            il = sp.tile([1, BL], f32, tag="il")
            nc.vector.reciprocal(il, lsum)
            wn = sp.tile([1, BL, NCH], f32, tag="wn")
            for b in range(BL):
                nc.vector.tensor_scalar_mul(wn[:, b, :], w[:, b, :],
                                            il[:, b : b + 1])
            wbc = ebp.tile([128, BL, NCH], f32, tag="wbc")
            nc.gpsimd.partition_broadcast(
                wbc.rearrange("p b n -> p (b n)"),
                wn.rearrange("p b n -> p (b n)"))
            sfin = sap.tile([128, KT, BL], f32, tag="sacc")
            for i in range(NCH):
                for b in range(BL):
                    if i == 0:
                        nc.vector.tensor_scalar_mul(
                            sfin[:, :, b], saccs[0][:, :, b],
                            wbc[:, b, 0:1])
                    else:
                        tmp = sp.tile([128, KT], f32, tag="ftmp")
                        nc.vector.tensor_scalar_mul(
                            tmp, saccs[i][:, :, b], wbc[:, b, i : i + 1])
                        nc.vector.tensor_add(
                            sfin[:, :, b], sfin[:, :, b], tmp)
            t_ps = py.tile([KT * BL, 128], f32, tag="y")
            nc.tensor.transpose(
                t_ps, sfin.rearrange("p t b -> p (t b)"), ident)
            t_sb = sp.tile([KT * BL, 128], f32, tag="tsb")
            nc.vector.tensor_copy(t_sb, t_ps)
            for t in range(KT):
                nc.sync.dma_start(
                    out=out[:, t * 128 : (t + 1) * 128],
                    in_=t_sb[t * BL : (t + 1) * BL, :])

    nc.compile()
    _split_pe_waits(nc)
    return nc


def _split_pe_waits(nc):
    """TRN2 PE instructions (S3_LW encoding) take a single sync-wait slot.
    Bacc's legalization misses some Matmults; hoist excess waits onto
    dedicated PE NoOps inserted directly before the offender."""
    for f in nc.m.functions:
        for bb in f.blocks:
            insts = bb.instructions
            i = 0
            while i < len(insts):
                ins = insts[i]
                if type(ins).__name__ in ("InstMatmult", "InstNoOp") and \
                        ins.engine == mybir.EngineType.PE:
                    si = ins.sync_info
                    if si is not None and len(si.on_wait) > 1:
                        extra, keep = si.on_wait[:-1], si.on_wait[-1:]
                        for w in extra:
                            nop = mybir.InstNoOp(
                                name=nc.get_next_instruction_name(),
                                ins=[], outs=[])
                            nop.engine = ins.engine
                            nop.sync_info = mybir.SyncInfo(
                                on_wait=[w], on_update=[])
                            nc.register_instruction(nop)
                            insts.insert(i, nop)
                            i += 1
                        si.on_wait = keep
                i += 1


def prep_inputs(h_i, h_t, mask, W, b, u, S=S, H=H, A=A, C=256, clist=None):
    """Shard + lay out the full inputs for the 8 cores."""
    h_i = np.asarray(h_i, np.float32)
    h_t = np.asarray(h_t, np.float32)
    mask = np.asarray(mask, bool)
    W = np.asarray(W, np.float32)
    b = np.asarray(b, np.float32)
    u = np.asarray(u, np.float32)

    KT = H // 128
    AT = A // 128
    if clist is None:
        clist = [C] * (S // C)
    offs = np.concatenate([[0], np.cumsum(clist)]).astype(int)
    w1t = np.ascontiguousarray(W[:, :H].T)                      # [H, A]
    cb = h_t @ W[:, H:].T + b                                   # [B, A]
    cb2s = np.ascontiguousarray(
        cb.reshape(B, AT, 128).transpose(2, 1, 0))              # [128, AT, B]
    u2 = np.ascontiguousarray(u[:, 0].reshape(AT, 128).T)       # [128, AT]

    pos = np.clip(np.cumsum(mask.astype(np.int64), axis=0) - 1, 0, None)
    onehot = (np.arange(B)[None, :, None] == pos[:, None, :]) & mask[:, None, :]
    selall = onehot.astype(np.float32)                          # [B, B, S]
    negall = np.where(mask, np.float32(0), NEG).astype(np.float32)  # [B, S]
    sel33 = np.concatenate([selall, negall[:, None, :]], axis=1)  # [B, B+1, S]

    in_maps = []
    for c in range(NCORES):
        bs = slice(c * BL, (c + 1) * BL)
        # hi5[p, block_i ++ (t, b, s)] = h_i[b, off_i+s, t*128+p]
        hcf = h_i[bs].reshape(BL, S, KT, 128)
        blocks = []
        for ci, off in zip(clist, offs[:-1]):
            hc = hcf[:, off : off + ci]                     # [BL, ci, KT, 128]
            blocks.append(hc.transpose(3, 2, 0, 1).reshape(128, KT * BL * ci))
        hi5 = np.ascontiguousarray(np.concatenate(blocks, axis=1))
        in_maps.append({
            "hi5": hi5,
            "w1t": w1t,
            "cb2": np.ascontiguousarray(cb2s[:, :, bs]),
            "u2": u2,
            "sel": np.ascontiguousarray(sel33[bs].transpose(1, 0, 2)),
        })
    return in_maps


_NC_CACHE = {}


CLIST = [128, 128] + [256] * 7


def _get_nc():
    if "nc" not in _NC_CACHE:
        _NC_CACHE["nc"] = build_kernel(clist=CLIST)
    return _NC_CACHE["nc"]


def kernel(h_i, h_t, mask, W, b, u):
    nc = _get_nc()
    in_maps = prep_inputs(h_i, h_t, mask, W, b, u, clist=CLIST)
    res = run_bass_kernel_spmd(nc, in_maps, list(range(NCORES)))
    return np.concatenate([res.results[c]["out"] for c in range(NCORES)],
                          axis=0)



# revision 3
# speedup vs baseline: 1.3438x; 1.3438x over previous
"""Trainium2 Bass kernel for nn_Attention_b (tanh-attention with masked_scatter).

Data-parallel over batch: each of 8 NeuronCores owns 4 batches. Heavy
operands travel in fp16 (score error ~7e-4 rel, same as fp32 baseline):
halves HBM traffic and doubles DVE throughput.

Per core, chunked over the sequence with a K-deep software pipeline so the
per-chunk AllGather of raw scores never blocks an engine queue:
  phase 1  z = W1 @ h_i (+cb via act bias); m = tanh(z); y = u . m
           AllGather y chunk (fp16) across the 8 cores
  phase 2  (K chunks later) masked_scatter selection via one-hot matmul
           + chunk-local softmax stats
  phase 3  sacc_i += e * h_i  (DVE fused multiply-reduce, fp16 inputs)
  final    flash-style combine of chunk partials, transpose, store
"""
import sys

for _p in ("/opt/trn_rl_repo",):
    if _p not in sys.path:
        sys.path.insert(0, _p)

import numpy as np

import concourse.bacc as bacc
import concourse.tile as tile
from concourse import mybir
from concourse.bass_utils import run_bass_kernel_spmd
from concourse.dve_ops import TENSOR_TENSOR_REDUCE
from concourse.masks import make_identity

NCORES = 8
B, S, H, A = 32, 2048, 1024, 256
BL = B // NCORES          # local batches per core
NEG = np.float32(-60000.0)   # fits fp16; exp(-6e4-max) == 0 regardless

f32 = mybir.dt.float32
f16 = mybir.dt.float16


def build_kernel(S=S, H=H, A=A, clist=None, K=4, hi_bufs=6):
    KT = H // 128             # contraction tiles
    AT = A // 128             # score tiles
    if clist is None:
        clist = [128, 128] + [256] * 7
    offs = np.concatenate([[0], np.cumsum(clist)]).tolist()
    NCH = len(clist)
    assert offs[-1] == S and H % 128 == 0 and A % 128 == 0

    nc = bacc.Bacc("TRN2", target_bir_lowering=False, debug=False,
                   num_devices=NCORES)

    hi5 = nc.declare_dram_parameter("hi5", [128, KT * BL * S], f16,
                                    isOutput=False)
    w1t = nc.declare_dram_parameter("w1t", [H, A], f16, isOutput=False)
    cb2 = nc.declare_dram_parameter("cb2", [128, AT, BL], f32, isOutput=False)
    u2 = nc.declare_dram_parameter("u2", [128, AT], f16, isOutput=False)
    sel = nc.declare_dram_parameter("sel", [B + 1, BL, S], f16,
                                    isOutput=False)
    out = nc.declare_dram_parameter("out", [BL, H], f32, isOutput=True)

    with tile.TileContext(nc) as tc:
        with (
            tc.tile_pool(name="consts", bufs=1) as cp,
            tc.tile_pool(name="hi", bufs=hi_bufs) as hip,
            tc.tile_pool(name="m", bufs=2) as mp,
            tc.tile_pool(name="small", bufs=K + 2) as sp,
            tc.tile_pool(name="tiny", bufs=3) as tp,
            tc.tile_pool(name="ebc", bufs=2) as ebp,
            tc.tile_pool(name="sacc", bufs=2) as sap,
            tc.tile_pool(name="pz", bufs=2, space="PSUM") as pz,
            tc.tile_pool(name="py", bufs=2, space="PSUM") as py,
            tc.tile_pool(name="dram", bufs=NCH, space="DRAM") as dp,
        ):
            # ---- preload replicated constants
            w1_sb = cp.tile([128, KT, A], f16)
            nc.sync.dma_start(
                out=w1_sb, in_=w1t.rearrange("(t p) a -> p t a", p=128))
            u_sb = cp.tile([128, AT], f16)
            nc.sync.dma_start(out=u_sb, in_=u2[:, :])
            cb_sb = cp.tile([128, AT, BL], f32)
            nc.sync.dma_start(out=cb_sb, in_=cb2[:, :, :])
            ident = cp.tile([128, 128], f32)
            make_identity(nc, ident)
            ones_sb = cp.tile([B + 1, 1], f16)
            nc.vector.memset(ones_sb, 1.0)
            wup = cp.tile([128, 512], f16)
            nc.vector.memset(wup, 0.0)

            # ---- warm-up: align cores with a tiny collective; ramp PE
            wg_in = dp.tile([1], f16, tag="wgin")
            nc.scalar.dma_start(out=wg_in.rearrange("(o n) -> o n", o=1),
                                in_=ones_sb[0:1, 0:1])
            wg_out = dp.tile([8], f16, tag="wgout", addr_space="Shared")
            nc.gpsimd.collective_compute(
                "AllGather", mybir.AluOpType.bypass,
                ins=[wg_in[:]], outs=[wg_out[:]],
                replica_groups=[list(range(NCORES))],
            )
            wup_ps = pz.tile([128, BL, 256], f32, tag="z")
            for i in range(8):
                nc.tensor.matmul(
                    wup_ps.rearrange("p b c -> p (b c)")[:, 0:512],
                    wup[:, 0:128], wup[:, 0:512],
                    start=True, stop=True)

            # ---- per-chunk softmax stats (combined once at the end)
            mall = cp.tile([1, BL, NCH], f32)
            lall = cp.tile([1, BL, NCH], f32)
            saccs = [cp.tile([128, KT, BL], f32, name=f"sacc{i}")
                     for i in range(NCH)]

            def phase1(i):
                Ci, off = clist[i], offs[i]
                hi_sb = hip.tile([128, KT, BL, Ci], f16, tag="hi")
                nc.sync.dma_start(
                    out=hi_sb.rearrange("p t b s -> p (t b s)"),
                    in_=hi5[:, KT * BL * off : KT * BL * (off + Ci)])
                sel_c = sp.tile([B + 1, BL, Ci], f16, tag="selc")
                nc.scalar.dma_start(out=sel_c, in_=sel[:, :, off : off + Ci])
                m_r = mp.tile([128, AT, BL, Ci], f16, tag="m")
                for at in range(AT):
                    z_ps = pz.tile([128, BL, Ci], f32, tag="z")
                    for r in range(BL // 2):
                        for kt in range(KT):
                            nc.tensor.matmul(
                                z_ps[:, 2 * r : 2 * r + 2, :],
                                w1_sb[:, kt, at * 128 : (at + 1) * 128],
                                hi_sb[:, kt, 2 * r : 2 * r + 2, :],
                                start=(kt == 0), stop=(kt == KT - 1),
                            )
                    for b in range(BL):
                        nc.scalar.activation(
                            out=m_r[:, at, b, :], in_=z_ps[:, b, :],
                            func=mybir.ActivationFunctionType.Tanh,
                            bias=cb_sb[:, at, b : b + 1], scale=1.0,
                        )
                y_ps = py.tile([1, BL, Ci], f32, tag="y")
                for r in range(BL // 2):
                    for at in range(AT):
                        nc.tensor.matmul(
                            y_ps[:, 2 * r : 2 * r + 2, :],
                            u_sb[:, at : at + 1],
                            m_r[:, at, 2 * r : 2 * r + 2, :],
                            start=(at == 0), stop=(at == AT - 1),
                        )
                y_sb = tp.tile([1, BL, Ci], f16, tag="ysb")
                nc.scalar.activation(out=y_sb, in_=y_ps,
                                     func=mybir.ActivationFunctionType.Copy)

                ag_in = dp.tile([BL * Ci], f16, tag="agin")
                nc.scalar.dma_start(
                    out=ag_in.rearrange("(o n) -> o n", o=1),
                    in_=y_sb.rearrange("p b s -> p (b s)"))
                ag_out = dp.tile([B * Ci], f16, tag="agout",
                                 addr_space="Shared")
                nc.gpsimd.collective_compute(
                    "AllGather", mybir.AluOpType.bypass,
                    ins=[ag_in[:]], outs=[ag_out[:]],
                    replica_groups=[list(range(NCORES))],
                )
                return dict(hi_sb=hi_sb, sel_c=sel_c, ag_out=ag_out,
                            i=i, Ci=Ci)

            def phase2(c):
                i, Ci = c["i"], c["Ci"]
                sel_c = c["sel_c"]
                # gather collective result (AG finished K chunks ago)
                y32 = tp.tile([B + 1, Ci], f16, tag="y32")
                nc.gpsimd.memset(y32[B : B + 1, :], 1.0)
                nc.scalar.dma_start(
                    out=y32[:B, :],
                    in_=c["ag_out"].rearrange("(j s) -> j s", s=Ci))
                # masked_scatter selection: one-hot rows (plus a NEG mask
                # row) dotted with [y; 1]
                bt_ps = py.tile([1, BL, Ci], f32, tag="y")
                selY = tp.tile([B + 1, BL, Ci], f16, tag="selY")
                nc.vector.tensor_mul(
                    selY, sel_c,
                    y32.rearrange("j (o s) -> j o s", o=1)
                       .broadcast_to([B + 1, BL, Ci]))
                for hf in range(2):
                    nc.tensor.matmul(
                        bt_ps[:, 2 * hf : 2 * hf + 2, :], ones_sb,
                        selY[:, 2 * hf : 2 * hf + 2, :],
                        start=True, stop=True)

                # chunk-local max -> no cross-chunk recurrence
                cmax = tp.tile([1, BL], f32, tag="cmax")
                nc.vector.tensor_reduce(
                    out=cmax.rearrange("p (b o) -> p b o", o=1), in_=bt_ps,
                    axis=mybir.AxisListType.X, op=mybir.AluOpType.max)
                nc.vector.tensor_copy(mall[:, :, i], cmax)
                nmnew = tp.tile([1, BL], f32, tag="nmnew")
                nc.vector.tensor_scalar_mul(nmnew, cmax, -1.0)
                e4 = tp.tile([1, BL, Ci], f16, tag="e4")
                for b in range(BL):
                    nc.scalar.activation(
                        out=e4[:, b, :], in_=bt_ps[:, b, :],
                        func=mybir.ActivationFunctionType.Exp,
                        bias=nmnew[:, b : b + 1], scale=1.0,
                        accum_out=lall[:, b, i : i + 1])
                e_bc = ebp.tile([128, BL, Ci], f16, tag="ebc")
                nc.gpsimd.partition_broadcast(
                    e_bc.rearrange("p b s -> p (b s)"),
                    e4.rearrange("p b s -> p (b s)"))
                c["ebc"] = e_bc

            def phase3(c):
                i, Ci = c["i"], c["Ci"]
                sacc_i = saccs[i]
                ttr_scr = tp.tile([128, 1], f16, tag="ttrscr")
                hi_sb = c["hi_sb"]
                e_bc_all = c["ebc"]
                for b in range(BL):
                    e_bc = e_bc_all[:, b, :]
                    for kt in range(KT):
                        nc.vector._custom_dve(
                            TENSOR_TENSOR_REDUCE,
                            out=ttr_scr.broadcast_to([128, Ci]),
                            in0=hi_sb[:, kt, b, :],
                            in1=e_bc,
                            s0=0.0, s1=1.0,
                            accum_out=sacc_i[:, kt, b : b + 1],
                        )

            carries = []
            for i in range(NCH):
                carries.append(phase1(i))
                if len(carries) > K:
                    c = carries.pop(0)
                    phase2(c)
                    phase3(c)
            while carries:
                c = carries.pop(0)
                phase2(c)
                phase3(c)

            # ---- finalize: combine chunk partials, divide, transpose, store
            M = tp.tile([1, BL], f32, tag="cmax")
            nc.vector.tensor_reduce(
                out=M.rearrange("p (b o) -> p b o", o=1), in_=mall,
                axis=mybir.AxisListType.X, op=mybir.AluOpType.max)
            nM = tp.tile([1, BL], f32, tag="nmnew")
            nc.vector.tensor_scalar_mul(nM, M, -1.0)
            w = tp.tile([1, BL, NCH], f32, tag="w")
            for b in range(BL):
                nc.scalar.activation(
                    out=w[:, b, :], in_=mall[:, b, :],
                    func=mybir.ActivationFunctionType.Exp,
                    bias=nM[:, b : b + 1], scale=1.0)
            wl = tp.tile([1, BL, NCH], f32, tag="wl")
            nc.vector.tensor_mul(wl, w, lall)
            lsum = tp.tile([1, BL], f32, tag="lsum")
            nc.vector.tensor_reduce(
                out=lsum.rearrange("p (b o) -> p b o", o=1), in_=wl,
                axis=mybir.AxisListType.X, op=mybir.AluOpType.add)
            il = tp.tile([1, BL], f32, tag="il")
            nc.vector.reciprocal(il, lsum)
            wn = tp.tile([1, BL, NCH], f32, tag="wn")
            for b in range(BL):
                nc.vector.tensor_scalar_mul(wn[:, b, :], w[:, b, :],
                                            il[:, b : b + 1])
            wbc = ebp.tile([128, BL, NCH], f32, tag="wbcf")
            nc.gpsimd.partition_broadcast(
                wbc.rearrange("p b n -> p (b n)"),
                wn.rearrange("p b n -> p (b n)"))
            sfin = sap.tile([128, KT, BL], f32, tag="sacc")
            for i in range(NCH):
                for b in range(BL):
                    if i == 0:
                        nc.vector.tensor_scalar_mul(
                            sfin[:, :, b], saccs[0][:, :, b],
                            wbc[:, b, 0:1])
                    else:
                        tmp = tp.tile([128, KT], f32, tag="ftmp")
                        nc.vector.tensor_scalar_mul(
                            tmp, saccs[i][:, :, b], wbc[:, b, i : i + 1])
                        nc.vector.tensor_add(
                            sfin[:, :, b], sfin[:, :, b], tmp)
            t_ps = py.tile([KT * BL, 128], f32, tag="y")
            nc.tensor.transpose(
                t_ps, sfin.rearrange("p t b -> p (t b)"), ident)
            t_sb = tp.tile([KT * BL, 128], f32, tag="tsb")
            nc.vector.tensor_copy(t_sb, t_ps)
            for t in range(KT):
                nc.sync.dma_start(
                    out=out[:, t * 128 : (t + 1) * 128],
                    in_=t_sb[t * BL : (t + 1) * BL, :])

    nc.compile()
    _split_pe_waits(nc)
    return nc


def _split_pe_waits(nc):
    """TRN2 PE instructions (S3_LW encoding) take a single sync-wait slot.
    Bacc's legalization misses some Matmults; hoist excess waits onto
    dedicated PE NoOps inserted directly before the offender."""
    for f in nc.m.functions:
        for bb in f.blocks:
            insts = bb.instructions
            i = 0
            while i < len(insts):
                ins = insts[i]
                if type(ins).__name__ in ("InstMatmult", "InstNoOp") and \
                        ins.engine == mybir.EngineType.PE:
                    si = ins.sync_info
                    if si is not None and len(si.on_wait) > 1:
                        extra, keep = si.on_wait[:-1], si.on_wait[-1:]
                        for w in extra:
                            nop = mybir.InstNoOp(
                                name=nc.get_next_instruction_name(),
                                ins=[], outs=[])
                            nop.engine = ins.engine
                            nop.sync_info = mybir.SyncInfo(
                                on_wait=[w], on_update=[])
                            nc.register_instruction(nop)
                            insts.insert(i, nop)
                            i += 1
                        si.on_wait = keep
                i += 1


def prep_inputs(h_i, h_t, mask, W, b, u, S=S, H=H, A=A, clist=None):
    """Shard + lay out the full inputs for the 8 cores (heavy data fp16)."""
    h_i = np.asarray(h_i, np.float32)
    h_t = np.asarray(h_t, np.float32)
    mask = np.asarray(mask, bool)
    W = np.asarray(W, np.float32)
    b = np.asarray(b, np.float32)
    u = np.asarray(u, np.float32)

    KT = H // 128
    AT = A // 128
    if clist is None:
        clist = [128, 128] + [256] * 7
    offs = np.concatenate([[0], np.cumsum(clist)]).astype(int)
    w1t = np.ascontiguousarray(W[:, :H].T).astype(np.float16)   # [H, A]
    cb = h_t @ W[:, H:].T + b                                   # [B, A]
    cb2s = np.ascontiguousarray(
        cb.reshape(B, AT, 128).transpose(2, 1, 0))              # [128, AT, B]
    u2 = np.ascontiguousarray(
        u[:, 0].reshape(AT, 128).T).astype(np.float16)          # [128, AT]

    pos = np.clip(np.cumsum(mask.astype(np.int64), axis=0) - 1, 0, None)
    onehot = (np.arange(B)[None, :, None] == pos[:, None, :]) & mask[:, None, :]
    selall = onehot.astype(np.float16)                          # [B, B, S]
    negall = np.where(mask, np.float16(0), np.float16(NEG))     # [B, S]
    sel33 = np.concatenate([selall, negall[:, None, :]], axis=1)  # [B, B+1, S]

    h16 = h_i.astype(np.float16)
    in_maps = []
    for c in range(NCORES):
        bs = slice(c * BL, (c + 1) * BL)
        # hi5[p, block_i ++ (t, b, s)] = h_i[b, off_i+s, t*128+p]
        hcf = h16[bs].reshape(BL, S, KT, 128)
        blocks = []
        for ci, off in zip(clist, offs[:-1]):
            hc = hcf[:, off : off + ci]                     # [BL, ci, KT, 128]
            blocks.append(hc.transpose(3, 2, 0, 1).reshape(128, KT * BL * ci))
        hi5 = np.ascontiguousarray(np.concatenate(blocks, axis=1))
        in_maps.append({
            "hi5": hi5,
            "w1t": w1t,
            "cb2": np.ascontiguousarray(cb2s[:, :, bs]),
            "u2": u2,
            "sel": np.ascontiguousarray(sel33[bs].transpose(1, 0, 2)),
        })
    return in_maps


_NC_CACHE = {}


CLIST = [128, 128] + [256] * 7


def _get_nc():
    if "nc" not in _NC_CACHE:
        _NC_CACHE["nc"] = build_kernel(clist=CLIST)
    return _NC_CACHE["nc"]


def kernel(h_i, h_t, mask, W, b, u):
    nc = _get_nc()
    in_maps = prep_inputs(h_i, h_t, mask, W, b, u, clist=CLIST)
    res = run_bass_kernel_spmd(nc, in_maps, list(range(NCORES)))
    return np.concatenate([res.results[c]["out"] for c in range(NCORES)],
                          axis=0)
